# revision 4
# baseline (speedup 1.0000x reference)
"""Trainium2 Bass kernel for nn_Attention_35588099015470.

Full attention block: LoRA linears (folded host-side) + RoPE + causal SDPA +
output projection. B=2 T=2048 C=2048 H=16 D=128, fp32 in/out.

Sharding: hybrid 2 (batch) x 4 (head-group). Core c handles batch c//4 and
heads 4*(c%4)..4*(c%4)+3. Each core loads only its batch's activations
(half the x traffic of pure head-parallel) and computes q/k/v for its four
heads over the full sequence.

All matmul operands are bf16 (host-cast; fp32 PSUM accumulation), which
halves HBM traffic and SBUF footprint at the same PE rate as fp32r, letting
q/k/v stay SBUF-resident between the projection and attention phases (no
DRAM round-trip). v is computed directly in natural [token, feat] layout by
swapping stationary/moving operands, so no PE transposes are needed.

Attention runs in [key, query] score layout: softmax denominators come from
an all-ones stationary matmul, the causal mask is applied as a 0/1 multiply
on the vector engine (cheaper than PE mask-add matmuls), and normalization
is a deferred per-row reciprocal multiply.

The output re-shard is one AllToAll per 512-query round (all 8 cores, mixed
batches): each core ends up with 64 tokens of each batch per round with all
2048 features, and the 2048x2048 output projection for those tokens runs
while the next round's attention occupies the PE — only the last round's
projection sits on the tail.

Biases are guaranteed zero by the problem's setup_inputs and the mask is the
causal tril; if either assumption is violated at runtime we fall back to a
host reference implementation so the kernel stays correct on any input.
"""
import sys

sys.path.insert(0, "/opt/trn_rl_repo")

import numpy as np
import ml_dtypes
from contextlib import ExitStack

import concourse.tile as tile
from concourse import bacc, mybir
from concourse.bass_utils import run_bass_kernel_spmd

dt = mybir.dt
BF = dt.bfloat16

B, T, C, H, R = 2, 2048, 2048, 16, 8
D = C // H            # 128
NCORES = 8
HPC = 4               # heads per core
P = 128
KC = C // P           # 16 contraction chunks
QT = T // 512         # 4 query rounds
SCALE = 1.0 / float(np.sqrt(D))

_PROGRAM = None


def _build_program():
    nc = bacc.Bacc("TRN2", target_bir_lowering=False, debug=False,
                   num_devices=NCORES)

    xT_d = nc.dram_tensor("xT", [C, T], BF, kind="ExternalInput")
    wqT_d = nc.dram_tensor("wqT", [C, HPC * D], BF, kind="ExternalInput")
    wkT_d = nc.dram_tensor("wkT", [C, HPC * D], BF, kind="ExternalInput")
    wvT_d = nc.dram_tensor("wvT", [C, HPC * D], BF, kind="ExternalInput")
    pwB_d = nc.dram_tensor("pwB", [P, KC, KC, P], BF, kind="ExternalInput")
    cosA_d = nc.dram_tensor("cosA", [P, T], dt.float32, kind="ExternalInput")
    sinA_d = nc.dram_tensor("sinA", [P, T], dt.float32, kind="ExternalInput")
    binm_d = nc.dram_tensor("binm", [4, P, 512], BF, kind="ExternalInput")

    outT_d = nc.dram_tensor("outT", [C, 512], dt.float32, kind="ExternalOutput")

    with tile.TileContext(nc) as tc, ExitStack() as ctx:
        dram = ctx.enter_context(tc.tile_pool(name="dram", bufs=1, space="DRAM"))
        # A2A staging: one collective per 512-query round
        chs = [dram.tile([NCORES, HPC * D, 64], BF, name=f"ch_{r}")
               for r in range(QT)]
        yos = [dram.tile([NCORES, HPC * D, 64], BF, name=f"yo_{r}")
               for r in range(QT)]

        cst = ctx.enter_context(tc.tile_pool(name="cst", bufs=1))
        kvp = ctx.enter_context(tc.tile_pool(name="kvp", bufs=1))

        ones_f = cst.tile([P, P], dt.float32, name="ones_f")
        nc.any.memset(ones_f[:], 1.0)
        ones_r = cst.tile([P, P], BF, name="ones_r")
        nc.vector.tensor_copy(ones_r[:], ones_f[:])
        binm = cst.tile([P, 4, 512], BF, name="binm")

        qsb = kvp.tile([P, HPC, T], BF, name="qsb")
        ksb = kvp.tile([P, HPC, T], BF, name="ksb")
        vsb = kvp.tile([P, KC, HPC * D], BF, name="vsb")

        # ---------------- Phase A: q/k/v projections + RoPE -----------------
        with tc.tile_pool(name="pa_w", bufs=1) as wp, \
             tc.tile_pool(name="pa_x", bufs=2) as xp, \
             tc.tile_pool(name="pa_cs", bufs=1) as csp, \
             tc.tile_pool(name="pa_tmp", bufs=3) as tp, \
             tc.tile_pool(name="pa_ps", bufs=1, space="PSUM") as pp:

            xT_view = xT_d.ap().rearrange("(a p) t -> p a t", p=P)
            w_sbs = {}
            for nm in ("q", "k", "v"):
                w_sbs[nm] = wp.tile([P, KC, HPC * D], BF, name=f"w{nm}_sb")
            # DMA order: wq + first x tile first so the PE starts ASAP
            wview = wqT_d.ap().rearrange("(a p) m -> p a m", p=P)
            for g in range(4):
                nc.sync.dma_start(w_sbs["q"][:, g * 4:(g + 1) * 4, :],
                                  wview[:, g * 4:(g + 1) * 4, :])
            xt0 = xp.tile([P, KC, 512], BF, tag="xt", name="xt_0")
            for g in range(4):
                nc.sync.dma_start(xt0[:, g * 4:(g + 1) * 4, :],
                                  xT_view[:, g * 4:(g + 1) * 4, 0:512])
            cosA = csp.tile([P, T], dt.float32, name="cosA")
            nc.sync.dma_start(cosA[:], cosA_d.ap())
            sinA = csp.tile([P, T], dt.float32, name="sinA")
            nc.sync.dma_start(sinA[:], sinA_d.ap())
            for nm, wd in (("k", wkT_d), ("v", wvT_d)):
                wview = wd.ap().rearrange("(a p) m -> p a m", p=P)
                for g in range(4):
                    nc.sync.dma_start(w_sbs[nm][:, g * 4:(g + 1) * 4, :],
                                      wview[:, g * 4:(g + 1) * 4, :])
            for o in range(4):
                nc.sync.dma_start(binm[:, o, :], binm_d.ap()[o])
            wq_sb, wk_sb, wv_sb = w_sbs["q"], w_sbs["k"], w_sbs["v"]

            for tt in range(4):
                tsl = slice(tt * 512, (tt + 1) * 512)
                if tt == 0:
                    xt = xt0
                else:
                    xt = xp.tile([P, KC, 512], BF, tag="xt", name=f"xt_{tt}")
                    nc.sync.dma_start(xt[:], xT_view[:, :, tsl])
                for h in range(HPC):
                    for w_sb, dst in ((wq_sb, qsb), (wk_sb, ksb)):
                        ps = pp.tile([P, 512], dt.float32, tag="qk", bufs=4,
                                     name=f"psA_{tt}_{h}")
                        for kc in range(KC):
                            nc.tensor.matmul(
                                ps[:], w_sb[:, kc, h * P:(h + 1) * P],
                                xt[:, kc, :],
                                start=(kc == 0), stop=(kc == KC - 1))
                        # rope: y = raw*cosA + halfswap(raw)*sinA
                        t1 = tp.tile([P, 512], dt.float32, tag="t1",
                                     name=f"t1_{tt}_{h}")
                        nc.vector.tensor_mul(t1[:], ps[:], cosA[:, tsl])
                        t2 = tp.tile([P, 512], dt.float32, tag="t2",
                                     name=f"t2_{tt}_{h}")
                        nc.vector.tensor_mul(t2[0:64, :], ps[64:128, :],
                                             sinA[0:64, tsl])
                        nc.vector.tensor_mul(t2[64:128, :], ps[0:64, :],
                                             sinA[64:128, tsl])
                        nc.vector.tensor_add(dst[:, h, tsl], t1[:], t2[:])
                # v in natural [token, feat] layout: x chunk as stationary
                for cj in range(4):
                    ps = pp.tile([P, 512], dt.float32, tag="v", bufs=2,
                                 name=f"psV_{tt}_{cj}")
                    for kc in range(KC):
                        nc.tensor.matmul(
                            ps[:], xt[:, kc, cj * P:(cj + 1) * P],
                            wv_sb[:, kc, :],
                            start=(kc == 0), stop=(kc == KC - 1))
                    nc.scalar.copy(vsb[:, tt * 4 + cj, :], ps[:])

        # ---------------- Phase B: attention + Phase C: projection ----------
        with tc.tile_pool(name="pb_w", bufs=1) as pwp, \
             tc.tile_pool(name="pb_p", bufs=1) as ptp, \
             tc.tile_pool(name="pb_y", bufs=1) as yp, \
             tc.tile_pool(name="pc_y", bufs=1) as ycp, \
             tc.tile_pool(name="pc_o", bufs=1) as ocp, \
             tc.tile_pool(name="pb_ps", bufs=1, space="PSUM") as pb:

            pw_sb = pwp.tile([P, KC, KC, P], BF, name="pw_sb")
            for g in range(4):
                nc.sync.dma_start(pw_sb[:, g * 4:(g + 1) * 4, :, :],
                                  pwB_d.ap()[:, g * 4:(g + 1) * 4, :, :])

            outT_view = outT_d.ap().rearrange("(a p) t -> p a t", p=P)

            def emit_C(r):
                yAB = ycp.tile([P, KC, 128], BF, tag="yab", bufs=2,
                               name=f"yab_{r}")
                for i in range(NCORES):
                    hgi, bh = i % 4, i // 4
                    nc.sync.dma_start(
                        yAB[:, 4 * hgi:4 * hgi + 4, bh * 64:(bh + 1) * 64],
                        yos[r][i].rearrange("(a p) t -> p a t", p=P))
                out_sb = ocp.tile([P, KC, 128], dt.float32, tag="osb", bufs=2,
                                  name=f"osb_{r}")
                for co in range(KC):
                    pso = pb.tile([P, 128], dt.float32, tag="c", bufs=2,
                                  name=f"pso_{r}_{co}")
                    for kc in range(KC):
                        nc.tensor.matmul(pso[:], pw_sb[:, co, kc, :],
                                         yAB[:, kc, :],
                                         start=(kc == 0), stop=(kc == KC - 1))
                    nc.scalar.copy(out_sb[:, co, :], pso[:])
                nc.sync.dma_start(outT_view[:, :, r * 128:(r + 1) * 128],
                                  out_sb[:])

            for r in range(QT):
                n = 4 * (r + 1)
                for h in range(HPC):
                    qmv = qsb[:, h, r * 512:(r + 1) * 512]
                    smps = pb.tile([P, 512], dt.float32, tag="sm", bufs=2,
                                   name=f"sm_{r}_{h}")
                    pvps = pb.tile([P, 512], dt.float32, tag="pv", bufs=2,
                                   name=f"pv_{r}_{h}")
                    sc_tiles = {}

                    def emit_sc(jc, _h=h, _q=qmv, _r=r, _sc=sc_tiles):
                        ps = pb.tile([P, 512], dt.float32, tag="sc", bufs=2,
                                     name=f"sc_{_r}_{_h}_{jc}")
                        nc.tensor.matmul(ps[:], ksb[:, _h, jc * P:(jc + 1) * P],
                                         _q, start=True, stop=True)
                        _sc[jc] = ps

                    emit_sc(0)
                    if n > 1:
                        emit_sc(1)
                    for jc in range(n):
                        scps = sc_tiles.pop(jc)
                        pT = ptp.tile([P, 512], BF, tag="pT", bufs=4,
                                      name=f"pT_{r}_{h}_{jc}")
                        nc.scalar.activation(pT[:], scps[:],
                                             mybir.ActivationFunctionType.Exp,
                                             scale=SCALE)
                        if jc >= n - 4:
                            o = jc - (n - 4)
                            pTm = ptp.tile([P, 512], BF, tag="pTm", bufs=3,
                                           name=f"pTm_{r}_{h}_{jc}")
                            nc.vector.tensor_mul(pTm[:], pT[:], binm[:, o, :])
                            pTu = pTm
                        else:
                            pTu = pT
                        if jc + 2 < n:
                            emit_sc(jc + 2)
                        nc.tensor.matmul(smps[:], ones_r[:], pTu[:],
                                         start=(jc == 0), stop=(jc == n - 1))
                        nc.tensor.matmul(pvps[:], vsb[:, jc, h * P:(h + 1) * P],
                                         pTu[:],
                                         start=(jc == 0), stop=(jc == n - 1))

                    # deferred softmax normalization (overlaps next head's PE)
                    rec = yp.tile([1, 512], dt.float32, tag="rec", bufs=2,
                                  name=f"rec_{r}_{h}")
                    nc.vector.reciprocal(rec[:], smps[0:1, :])
                    bc = yp.tile([P, 512], dt.float32, tag="bc", bufs=2,
                                 name=f"bc_{r}_{h}")
                    nc.gpsimd.partition_broadcast(bc[:], rec[:])
                    yt = yp.tile([P, 512], BF, tag="yt", bufs=2,
                                 name=f"yt_{r}_{h}")
                    nc.vector.tensor_mul(yt[:], pvps[:], bc[:])
                    nc.sync.dma_start(
                        chs[r][:, h * P:(h + 1) * P, :]
                        .rearrange("s d t -> d s t"),
                        yt[:].rearrange("d (s t) -> d s t", s=NCORES))

                nc.gpsimd.collective_compute(
                    "AllToAll", mybir.AluOpType.bypass,
                    replica_groups=[list(range(NCORES))],
                    ins=[chs[r].opt()], outs=[yos[r].opt()],
                )
                if r >= 1:
                    emit_C(r - 1)
            emit_C(QT - 1)

    nc.compile()
    return nc


def _host_reference(x, weights, cos, sin, mask, use_lora):
    """Numpy fallback for inputs outside the optimized assumptions."""
    (q_w, q_b, q_A, q_B, k_w, k_b, k_A, k_B,
     v_w, v_b, v_A, v_B, p_w, p_b, p_A, p_B) = weights

    def lin(xx, w, b, A, Bm):
        out = xx @ w.T + b
        if use_lora:
            out = out + (xx @ A) @ Bm
        return out

    def rope(t):
        x1, x2 = t[..., ::2], t[..., 1::2]
        y = np.stack((x1 * cos - x2 * sin, x1 * sin + x2 * cos), axis=-1)
        return y.reshape(t.shape)

    Bs, Tl, Cd = x.shape
    q = lin(x, q_w, q_b, q_A, q_B).reshape(Bs, Tl, H, D).transpose(0, 2, 1, 3)
    k = lin(x, k_w, k_b, k_A, k_B).reshape(Bs, Tl, H, D).transpose(0, 2, 1, 3)
    v = lin(x, v_w, v_b, v_A, v_B).reshape(Bs, Tl, H, D).transpose(0, 2, 1, 3)
    q, k = rope(q), rope(k)
    s = np.einsum('bhqd,bhkd->bhqk', q, k) / np.sqrt(D)
    s = np.where(mask, s, -np.inf)
    s = s - s.max(axis=-1, keepdims=True)
    p = np.exp(s)
    p /= p.sum(axis=-1, keepdims=True)
    o = np.einsum('bhqk,bhkd->bhqd', p, v).transpose(0, 2, 1, 3).reshape(Bs, Tl, Cd)
    return lin(o, p_w, p_b, p_A, p_B).astype(np.float32)


def kernel(**inputs):
    x = np.asarray(inputs["x"], np.float32)
    cos = np.asarray(inputs["cos"], np.float32)
    sin = np.asarray(inputs["sin"], np.float32)
    mask = np.asarray(inputs["mask"])
    use_lora = int(np.asarray(inputs["use_lora"]))
    ws = {}
    for nm in ("q", "k", "v", "p"):
        for suf in ("w", "b", "A", "B"):
            ws[f"{nm}_{suf}"] = np.asarray(inputs[f"{nm}_{suf}"], np.float32)

    causal = bool((mask == np.tril(np.ones((T, T), bool))).all())
    zero_bias = all(not ws[f"{nm}_b"].any() for nm in ("q", "k", "v", "p"))
    if not (causal and zero_bias and x.shape == (B, T, C)):
        weights = tuple(ws[f"{nm}_{suf}"] for nm in ("q", "k", "v", "p")
                        for suf in ("w", "b", "A", "B"))
        return _host_reference(x, weights, cos, sin, mask, use_lora)

    bf = ml_dtypes.bfloat16

    # effective (LoRA-folded) transposed weights: out = x @ W_eff.T,
    # W_eff.T = w.T + A @ B
    effT = {}
    for nm in ("q", "k", "v", "p"):
        wt = ws[f"{nm}_w"].T.copy()
        if use_lora:
            wt += ws[f"{nm}_A"] @ ws[f"{nm}_B"]
        effT[nm] = wt

    # sigma: within each head reorder q/k out-features to [evens, odds] so
    # the rope pair-rotation becomes a partition half-swap
    perm = np.concatenate([np.arange(0, D, 2), np.arange(1, D, 2)])
    cosT = cos.T.astype(np.float32)          # [64, T]
    sinT = sin.T.astype(np.float32)
    cosA = np.vstack([cosT, cosT])           # [128, T]
    sinA = np.vstack([-sinT, sinT])

    # 0/1 causal mask for the 4 diagonal 128-key x 512-query offsets
    jj = np.arange(P)[:, None]
    qq = np.arange(512)[None, :]
    binm = np.stack([(jj + P * o <= qq) for o in range(4)]).astype(bf)

    # output projection weight, blocked [p_ci, co, kc, m]
    pwB = np.ascontiguousarray(
        effT["p"].reshape(KC, P, KC, P).transpose(1, 2, 0, 3)).astype(bf)

    global _PROGRAM
    if _PROGRAM is None:
        _PROGRAM = _build_program()
    nc = _PROGRAM

    in_maps = []
    for c in range(NCORES):
        b, hg = c // 4, c % 4
        cols = slice(hg * HPC * D, (hg + 1) * HPC * D)
        wqT = effT["q"][:, cols].copy()
        wkT = effT["k"][:, cols].copy()
        for hl in range(HPC):
            sl = slice(hl * D, (hl + 1) * D)
            wqT[:, sl] = wqT[:, sl][:, perm]
            wkT[:, sl] = wkT[:, sl][:, perm]
        in_maps.append({
            "xT": np.ascontiguousarray(x[b].T).astype(bf),
            "wqT": np.ascontiguousarray(wqT).astype(bf),
            "wkT": np.ascontiguousarray(wkT).astype(bf),
            "wvT": np.ascontiguousarray(effT["v"][:, cols]).astype(bf),
            "pwB": pwB,
            "cosA": cosA,
            "sinA": sinA,
            "binm": binm,
        })

    res = run_bass_kernel_spmd(nc, in_maps, list(range(NCORES)))

    out = np.empty((B, T, C), np.float32)
    for c in range(NCORES):
        oT = res.results[c]["outT"]                    # [2048, 4*2*64]
        blk = oT.reshape(C, QT, 2, 64)
        for r in range(QT):
            t0 = r * 512 + c * 64
            out[0, t0:t0 + 64, :] = blk[:, r, 0, :].T
            out[1, t0:t0 + 64, :] = blk[:, r, 1, :].T
    return out


# revision 5
# speedup vs baseline: 1.1007x; 1.1007x over previous
"""Trainium2 Bass kernel for nn_Attention_35588099015470.

Full attention block: LoRA linears (folded host-side) + RoPE + causal SDPA +
output projection. B=2 T=2048 C=2048 H=16 D=128, fp32 in/out.

Sharding: hybrid 2 (batch) x 4 (head-group). Core c handles batch c//4 and
heads 4*(c%4)..4*(c%4)+3, so each core loads only its batch's activations.

All matmul operands are bf16 (host-cast; fp32 PSUM accumulation) — same PE
rate as fp32r but half the HBM traffic and SBUF footprint, which lets q/k/v
stay SBUF-resident between projection and attention. v is produced directly
in natural [token, feat] layout by using the x tile as the matmul stationary,
so no PE transposes are needed.

Emission interleaves the three stages per 512-token tile —
A(t) projection+RoPE, att(t) causal attention, then the output-projection
round C(t-1) — so the per-round AllToAll (which re-shards head-parallel y to
token-parallel) completes in the shadow of the next tile's projection GEMMs
and the PE never waits on the collective. Attention keeps scores in
[key, query] layout: softmax denominators via an all-ones stationary matmul,
causal masking as a 0/1 vector-engine multiply, normalization as a deferred
reciprocal(approx)+broadcast+multiply. The C rounds keep the moving operand
512 wide (stationary = incoming activations, moving = projection weight) so
LDWEIGHTS stays pipelined behind the matmul stream.

Biases are guaranteed zero by the problem's setup_inputs and the mask is the
causal tril; if either assumption is violated at runtime we fall back to a
host reference implementation so the kernel stays correct on any input.
"""
import sys

sys.path.insert(0, "/opt/trn_rl_repo")

import numpy as np
import ml_dtypes
from contextlib import ExitStack

import concourse.tile as tile
from concourse import bacc, mybir
from concourse.bass_utils import run_bass_kernel_spmd

dt = mybir.dt
BF = dt.bfloat16

B, T, C, H, R = 2, 2048, 2048, 16, 8
D = C // H            # 128
NCORES = 8
HPC = 4               # heads per core
P = 128
KC = C // P           # 16 contraction chunks
QT = T // 512         # 4 query rounds / token tiles
SCALE = 1.0 / float(np.sqrt(D))

_PROGRAM = None


def _build_program():
    nc = bacc.Bacc("TRN2", target_bir_lowering=False, debug=False,
                   num_devices=NCORES)

    xT_d = nc.dram_tensor("xT", [C, T], BF, kind="ExternalInput")
    wqT_d = nc.dram_tensor("wqT", [C, HPC * D], BF, kind="ExternalInput")
    wkT_d = nc.dram_tensor("wkT", [C, HPC * D], BF, kind="ExternalInput")
    wvT_d = nc.dram_tensor("wvT", [C, HPC * D], BF, kind="ExternalInput")
    pwM_d = nc.dram_tensor("pwM", [P, KC, C], BF, kind="ExternalInput")
    cosA_d = nc.dram_tensor("cosA", [P, T], dt.float32, kind="ExternalInput")
    sinA_d = nc.dram_tensor("sinA", [P, T], dt.float32, kind="ExternalInput")
    binm_d = nc.dram_tensor("binm", [4, P, 512], BF, kind="ExternalInput")

    outN_d = nc.dram_tensor("outN", [512, C], dt.float32, kind="ExternalOutput")

    with tile.TileContext(nc) as tc, ExitStack() as ctx:
        dram = ctx.enter_context(tc.tile_pool(name="dram", bufs=1, space="DRAM"))
        chs = [dram.tile([NCORES, HPC * D, 64], BF, name=f"ch_{r}")
               for r in range(QT)]
        yos = [dram.tile([NCORES, HPC * D, 64], BF, name=f"yo_{r}")
               for r in range(QT)]

        cst = ctx.enter_context(tc.tile_pool(name="cst", bufs=1))
        kvp = ctx.enter_context(tc.tile_pool(name="kvp", bufs=1))
        wp = ctx.enter_context(tc.tile_pool(name="wp", bufs=1))
        xp = ctx.enter_context(tc.tile_pool(name="xp", bufs=1))
        csp = ctx.enter_context(tc.tile_pool(name="csp", bufs=1))
        qp = ctx.enter_context(tc.tile_pool(name="qp", bufs=2))
        tp = ctx.enter_context(tc.tile_pool(name="tp", bufs=2))
        ptp = ctx.enter_context(tc.tile_pool(name="ptp", bufs=1))
        yp = ctx.enter_context(tc.tile_pool(name="yp", bufs=2))
        ycp = ctx.enter_context(tc.tile_pool(name="ycp", bufs=1))
        ocp = ctx.enter_context(tc.tile_pool(name="ocp", bufs=2))
        pp = ctx.enter_context(tc.tile_pool(name="pp", bufs=1, space="PSUM"))

        ones_f = cst.tile([P, P], dt.float32, name="ones_f")
        nc.any.memset(ones_f[:], 1.0)
        ones_r = cst.tile([P, P], BF, name="ones_r")
        nc.vector.tensor_copy(ones_r[:], ones_f[:])
        binm = cst.tile([P, 4, 512], BF, name="binm")

        k_t = [kvp.tile([P, HPC, 512], BF, name=f"k_{t}") for t in range(QT)]
        v_t = [kvp.tile([P, 4, HPC * D], BF, name=f"v_{t}") for t in range(QT)]

        xT_view = xT_d.ap().rearrange("(a p) t -> p a t", p=P)
        wq_sb = wp.tile([P, KC, HPC * D], BF, name="wq_sb")
        wk_sb = wp.tile([P, KC, HPC * D], BF, name="wk_sb")
        wv_sb = wp.tile([P, KC, HPC * D], BF, name="wv_sb")
        pw_sb = wp.tile([P, KC, C], BF, name="pw_sb")

        # DMA order: wq + first x tile first so the PE starts ASAP
        wqv = wqT_d.ap().rearrange("(a p) m -> p a m", p=P)
        xt0 = xp.tile([P, KC, 512], BF, tag="xt", name="xt_0")
        for g in range(4):
            nc.sync.dma_start(wq_sb[:, g * 4:(g + 1) * 4, :],
                              wqv[:, g * 4:(g + 1) * 4, :])
            nc.sync.dma_start(xt0[:, g * 4:(g + 1) * 4, :],
                              xT_view[:, g * 4:(g + 1) * 4, 0:512])
        cc0 = csp.tile([P, 512], dt.float32, tag="cc", name="cc_0")
        nc.sync.dma_start(cc0[:], cosA_d.ap()[:, 0:512])
        ss0 = csp.tile([P, 512], dt.float32, tag="ss", name="ss_0")
        nc.sync.dma_start(ss0[:], sinA_d.ap()[:, 0:512])
        for w_sb, wd in ((wk_sb, wkT_d), (wv_sb, wvT_d)):
            wvw = wd.ap().rearrange("(a p) m -> p a m", p=P)
            for g in range(4):
                nc.sync.dma_start(w_sb[:, g * 4:(g + 1) * 4, :],
                                  wvw[:, g * 4:(g + 1) * 4, :])
        for o in range(4):
            nc.sync.dma_start(binm[:, o, :], binm_d.ap()[o])
        for g in range(4):
            nc.sync.dma_start(pw_sb[:, g * 4:(g + 1) * 4, :],
                              pwM_d.ap()[:, g * 4:(g + 1) * 4, :])

        def emit_C(r):
            yAB = ycp.tile([P, KC, 128], BF, tag="yab", bufs=1,
                           name=f"yab_{r}")
            for i in range(NCORES):
                hgi, bh = i % 4, i // 4
                nc.sync.dma_start(
                    yAB[:, 4 * hgi:4 * hgi + 4, bh * 64:(bh + 1) * 64],
                    yos[r][i].rearrange("(a p) t -> p a t", p=P))
            for cb in range(4):
                cps = pp.tile([P, 512], dt.float32, tag="c", bufs=1,
                              name=f"cps_{r}_{cb}")
                for kc in range(KC):
                    nc.tensor.matmul(cps[:], yAB[:, kc, :],
                                     pw_sb[:, kc, cb * 512:(cb + 1) * 512],
                                     start=(kc == 0), stop=(kc == KC - 1))
                osb = ocp.tile([P, 512], dt.float32, tag="osb", bufs=2,
                               name=f"osb_{r}_{cb}")
                nc.scalar.copy(osb[:], cps[:])
                nc.sync.dma_start(
                    outN_d.ap()[r * P:(r + 1) * P, cb * 512:(cb + 1) * 512],
                    osb[:])

        for tt in range(QT):
            tsl = slice(tt * 512, (tt + 1) * 512)
            # ---- A(tt): q/k/v projections + rope for this token tile ----
            if tt == 0:
                xt, cc, ss = xt0, cc0, ss0
            else:
                xt = xp.tile([P, KC, 512], BF, tag="xt", name=f"xt_{tt}")
                nc.sync.dma_start(xt[:], xT_view[:, :, tsl])
                cc = csp.tile([P, 512], dt.float32, tag="cc", name=f"cc_{tt}")
                nc.sync.dma_start(cc[:], cosA_d.ap()[:, tsl])
                ss = csp.tile([P, 512], dt.float32, tag="ss", name=f"ss_{tt}")
                nc.sync.dma_start(ss[:], sinA_d.ap()[:, tsl])
            qtile = qp.tile([P, HPC, 512], BF, tag="qt", name=f"q_{tt}")
            for h in range(HPC):
                for w_sb, dst in ((wq_sb, qtile), (wk_sb, k_t[tt])):
                    ps = pp.tile([P, 512], dt.float32, tag="a", bufs=2,
                                 name=f"psA_{tt}_{h}")
                    for kc in range(KC):
                        nc.tensor.matmul(
                            ps[:], w_sb[:, kc, h * P:(h + 1) * P],
                            xt[:, kc, :],
                            start=(kc == 0), stop=(kc == KC - 1))
                    # rope: y = raw*cosA + halfswap(raw)*sinA
                    t1 = tp.tile([P, 512], dt.float32, tag="t1",
                                 name=f"t1_{tt}_{h}")
                    nc.vector.tensor_mul(t1[:], ps[:], cc[:])
                    t2 = tp.tile([P, 512], dt.float32, tag="t2",
                                 name=f"t2_{tt}_{h}")
                    nc.vector.tensor_mul(t2[0:64, :], ps[64:128, :],
                                         ss[0:64, :])
                    nc.vector.tensor_mul(t2[64:128, :], ps[0:64, :],
                                         ss[64:128, :])
                    nc.vector.tensor_add(dst[:, h, :], t1[:], t2[:])
            for cj in range(4):
                ps = pp.tile([P, 512], dt.float32, tag="a", bufs=2,
                             name=f"psV_{tt}_{cj}")
                for kc in range(KC):
                    nc.tensor.matmul(
                        ps[:], xt[:, kc, cj * P:(cj + 1) * P],
                        wv_sb[:, kc, :],
                        start=(kc == 0), stop=(kc == KC - 1))
                nc.scalar.copy(v_t[tt][:, cj, :], ps[:])

            # ---- C(tt-1): output projection for the previous round ----
            if tt >= 1:
                emit_C(tt - 1)

            # ---- att(tt): causal attention for queries of this tile ----
            n = 4 * (tt + 1)
            for h in range(HPC):
                qmv = qtile[:, h, :]
                smps = pp.tile([P, 512], dt.float32, tag="sm", bufs=1,
                               name=f"sm_{tt}_{h}")
                pvps = pp.tile([P, 512], dt.float32, tag="pv", bufs=2,
                               name=f"pv_{tt}_{h}")
                sc_tiles = {}

                def emit_sc(jc, _h=h, _q=qmv, _tt=tt, _sc=sc_tiles):
                    ps = pp.tile([P, 512], dt.float32, tag="sc", bufs=2,
                                 name=f"sc_{_tt}_{_h}_{jc}")
                    nc.tensor.matmul(
                        ps[:], k_t[jc // 4][:, _h, (jc % 4) * P:(jc % 4 + 1) * P],
                        _q, start=True, stop=True)
                    _sc[jc] = ps

                emit_sc(0)
                if n > 1:
                    emit_sc(1)
                for jc in range(n):
                    scps = sc_tiles.pop(jc)
                    pT = ptp.tile([P, 512], BF, tag="pT", bufs=3,
                                  name=f"pT_{tt}_{h}_{jc}")
                    nc.scalar.activation(pT[:], scps[:],
                                         mybir.ActivationFunctionType.Exp,
                                         scale=SCALE)
                    if jc >= n - 4:
                        o = jc - (n - 4)
                        pTm = ptp.tile([P, 512], BF, tag="pTm", bufs=2,
                                       name=f"pTm_{tt}_{h}_{jc}")
                        nc.vector.tensor_mul(pTm[:], pT[:], binm[:, o, :])
                        pTu = pTm
                    else:
                        pTu = pT
                    if jc + 2 < n:
                        emit_sc(jc + 2)
                    nc.tensor.matmul(smps[:], ones_r[:], pTu[:],
                                     start=(jc == 0), stop=(jc == n - 1))
                    nc.tensor.matmul(pvps[:],
                                     v_t[jc // 4][:, jc % 4, h * P:(h + 1) * P],
                                     pTu[:],
                                     start=(jc == 0), stop=(jc == n - 1))

                # deferred softmax normalization
                rec = yp.tile([1, 512], dt.float32, tag="rec", bufs=2,
                              name=f"rec_{tt}_{h}")
                nc.vector.reciprocal_approx_fast(rec[:], smps[0:1, :])
                bc = yp.tile([P, 512], dt.float32, tag="bc", bufs=2,
                             name=f"bc_{tt}_{h}")
                nc.gpsimd.partition_broadcast(bc[:], rec[:])
                yt = yp.tile([P, 512], BF, tag="yt", bufs=2,
                             name=f"yt_{tt}_{h}")
                nc.vector.tensor_mul(yt[:], pvps[:], bc[:])
                nc.sync.dma_start(
                    chs[tt][:, h * P:(h + 1) * P, :]
                    .rearrange("s d t -> d s t"),
                    yt[:].rearrange("d (s t) -> d s t", s=NCORES))

            nc.gpsimd.collective_compute(
                "AllToAll", mybir.AluOpType.bypass,
                replica_groups=[list(range(NCORES))],
                ins=[chs[tt].opt()], outs=[yos[tt].opt()],
            )
        emit_C(QT - 1)

    nc.compile()
    return nc


def _host_reference(x, weights, cos, sin, mask, use_lora):
    """Numpy fallback for inputs outside the optimized assumptions."""
    (q_w, q_b, q_A, q_B, k_w, k_b, k_A, k_B,
     v_w, v_b, v_A, v_B, p_w, p_b, p_A, p_B) = weights

    def lin(xx, w, b, A, Bm):
        out = xx @ w.T + b
        if use_lora:
            out = out + (xx @ A) @ Bm
        return out

    def rope(t):
        x1, x2 = t[..., ::2], t[..., 1::2]
        y = np.stack((x1 * cos - x2 * sin, x1 * sin + x2 * cos), axis=-1)
        return y.reshape(t.shape)

    Bs, Tl, Cd = x.shape
    q = lin(x, q_w, q_b, q_A, q_B).reshape(Bs, Tl, H, D).transpose(0, 2, 1, 3)
    k = lin(x, k_w, k_b, k_A, k_B).reshape(Bs, Tl, H, D).transpose(0, 2, 1, 3)
    v = lin(x, v_w, v_b, v_A, v_B).reshape(Bs, Tl, H, D).transpose(0, 2, 1, 3)
    q, k = rope(q), rope(k)
    s = np.einsum('bhqd,bhkd->bhqk', q, k) / np.sqrt(D)
    s = np.where(mask, s, -np.inf)
    s = s - s.max(axis=-1, keepdims=True)
    p = np.exp(s)
    p /= p.sum(axis=-1, keepdims=True)
    o = np.einsum('bhqk,bhkd->bhqd', p, v).transpose(0, 2, 1, 3).reshape(Bs, Tl, Cd)
    return lin(o, p_w, p_b, p_A, p_B).astype(np.float32)


def kernel(**inputs):
    x = np.asarray(inputs["x"], np.float32)
    cos = np.asarray(inputs["cos"], np.float32)
    sin = np.asarray(inputs["sin"], np.float32)
    mask = np.asarray(inputs["mask"])
    use_lora = int(np.asarray(inputs["use_lora"]))
    ws = {}
    for nm in ("q", "k", "v", "p"):
        for suf in ("w", "b", "A", "B"):
            ws[f"{nm}_{suf}"] = np.asarray(inputs[f"{nm}_{suf}"], np.float32)

    causal = bool((mask == np.tril(np.ones((T, T), bool))).all())
    zero_bias = all(not ws[f"{nm}_b"].any() for nm in ("q", "k", "v", "p"))
    if not (causal and zero_bias and x.shape == (B, T, C)):
        weights = tuple(ws[f"{nm}_{suf}"] for nm in ("q", "k", "v", "p")
                        for suf in ("w", "b", "A", "B"))
        return _host_reference(x, weights, cos, sin, mask, use_lora)

    bf = ml_dtypes.bfloat16

    # effective (LoRA-folded) transposed weights: out = x @ W_eff.T,
    # W_eff.T = w.T + A @ B
    effT = {}
    for nm in ("q", "k", "v", "p"):
        wt = ws[f"{nm}_w"].T.copy()
        if use_lora:
            wt += ws[f"{nm}_A"] @ ws[f"{nm}_B"]
        effT[nm] = wt

    # sigma: within each head reorder q/k out-features to [evens, odds] so
    # the rope pair-rotation becomes a partition half-swap
    perm = np.concatenate([np.arange(0, D, 2), np.arange(1, D, 2)])
    cosT = cos.T.astype(np.float32)          # [64, T]
    sinT = sin.T.astype(np.float32)
    cosA = np.ascontiguousarray(np.vstack([cosT, cosT]))
    sinA = np.ascontiguousarray(np.vstack([-sinT, sinT]))

    # 0/1 causal mask for the 4 diagonal 128-key x 512-query offsets
    jj = np.arange(P)[:, None]
    qq = np.arange(512)[None, :]
    binm = np.stack([(jj + P * o <= qq) for o in range(4)]).astype(bf)

    # output projection weight, blocked [p_ci, kc, co]
    pwM = np.ascontiguousarray(
        effT["p"].reshape(KC, P, C).transpose(1, 0, 2)).astype(bf)

    global _PROGRAM
    if _PROGRAM is None:
        _PROGRAM = _build_program()
    nc = _PROGRAM

    in_maps = []
    for c in range(NCORES):
        b, hg = c // 4, c % 4
        cols = slice(hg * HPC * D, (hg + 1) * HPC * D)
        wqT = effT["q"][:, cols].copy()
        wkT = effT["k"][:, cols].copy()
        for hl in range(HPC):
            sl = slice(hl * D, (hl + 1) * D)
            wqT[:, sl] = wqT[:, sl][:, perm]
            wkT[:, sl] = wkT[:, sl][:, perm]
        in_maps.append({
            "xT": np.ascontiguousarray(x[b].T).astype(bf),
            "wqT": np.ascontiguousarray(wqT).astype(bf),
            "wkT": np.ascontiguousarray(wkT).astype(bf),
            "wvT": np.ascontiguousarray(effT["v"][:, cols]).astype(bf),
            "pwM": pwM,
            "cosA": cosA,
            "sinA": sinA,
            "binm": binm,
        })

    res = run_bass_kernel_spmd(nc, in_maps, list(range(NCORES)))

    out = np.empty((B, T, C), np.float32)
    for c in range(NCORES):
        oN = res.results[c]["outN"]                    # [512, 2048]
        blk = oN.reshape(QT, 2, 64, C)
        for r in range(QT):
            t0 = r * 512 + c * 64
            out[0, t0:t0 + 64, :] = blk[r, 0]
            out[1, t0:t0 + 64, :] = blk[r, 1]
    return out
